# revision 16
# baseline (speedup 1.0000x reference)
"""COMPER (gnn_message_passing) forward pass on 8 Trainium2 NeuronCores.

Strategy: data-parallel over the batch (1024 samples/core), embedding tables
and weights replicated. Per core everything runs in a "transposed" layout
(feature/hidden dim on SBUF partitions, batch rows on the free dim) so LSTM /
MLP weights are the stationary matmul operand.

Host prep (index arithmetic / input encoding only):
  - user_emb and item_emb concatenated into one table; node indices combined
    as node + (type==1)*(NU+1) so one indirect-DMA gather fetches node embs.
  - (rel, type) categorical inputs encoded one-hot (16 rows); their gate
    contribution uses a precomputed 16x512 table
    G[s] = rel_emb[r] @ W_ih_rel.T + type_emb[t] @ W_ih_type.T + bias,
    so type/rel embedding lookups + bias become a single K=16 matmul.
  - LSTM gate rows permuted to [i, f, o, g] so one fused sigmoid covers i,f,o.

Heavy matmuls (LSTM gates / attention) run as float32r (fp32 storage,
single-pass PE mode); the small softmax + output head stays full fp32.
"""

import numpy as np

B, P, L = 8192, 10, 3
E, H = 128, 128
NU, NI, NR = 200000, 100000, 8
NCORES = 8

_BUILD_CACHE = {}


# --------------------------------------------------------------------------
# device program
# --------------------------------------------------------------------------
def build_program(bloc=1024, chunk=512, num_devices=NCORES, reps=1):
    """Build (and bacc-compile) the single-core Bass program.

    bloc:  samples per core; chunk: rows (=sample,path pairs) per inner tile.
    Row order is p-major: r = p * bloc + b.  chunk must divide bloc.
    reps > 1 wraps the compute body in a runtime For_i loop (for timing).
    """
    key = (bloc, chunk, num_devices, reps)
    if key in _BUILD_CACHE:
        return _BUILD_CACHE[key]

    import concourse.bass as bass
    import concourse.bacc as bacc
    import concourse.mybir as mybir
    import concourse.tile as tile
    from concourse.bass import IndirectOffsetOnAxis
    from contextlib import ExitStack

    dt = mybir.dt
    f32, f32r, i32 = dt.float32, dt.float32r, dt.int32
    AF = mybir.ActivationFunctionType
    MUL = mybir.AluOpType.mult

    RT = chunk // 128            # row-tiles (of 128) per chunk
    NCHUNK = (P * bloc) // chunk  # chunks per core
    CPP = bloc // chunk           # chunks per path index p
    ROWS = P * bloc
    GCOLS = 3 * RT                # gather columns per chunk (L * row-tiles)
    CW = 3 * chunk                # node_eT columns per chunk

    def nsplit(total, maxn=512):
        out = []
        o = 0
        while o < total:
            n = min(maxn, total - o)
            out.append((o, n))
            o += n
        return out

    nc = bacc.Bacc("TRN2", target_bir_lowering=False, debug=False,
                   num_devices=num_devices)

    TBL = NU + NI + 2
    d_table = nc.dram_tensor("table", [TBL, E], f32, kind="ExternalInput")
    d_idx_node = nc.dram_tensor("idx_node", [128, NCHUNK * GCOLS], i32,
                                kind="ExternalInput")
    d_idx_user = nc.dram_tensor("idx_user", [128, bloc // 128], i32,
                                kind="ExternalInput")
    d_idx_item = nc.dram_tensor("idx_item", [128, bloc // 128], i32,
                                kind="ExternalInput")
    d_onehot = nc.dram_tensor("onehot", [16, NCHUNK * CW], f32r,
                              kind="ExternalInput")
    d_w_node = nc.dram_tensor("w_node", [E, 4 * H], f32r, kind="ExternalInput")
    d_g_combo = nc.dram_tensor("g_combo", [16, 4 * H], f32r,
                               kind="ExternalInput")
    d_w_hh = nc.dram_tensor("w_hh", [H, 4 * H], f32r, kind="ExternalInput")
    d_att_p = nc.dram_tensor("att_p", [H, 64], f32r, kind="ExternalInput")
    d_att_u = nc.dram_tensor("att_u", [E, 64], f32r, kind="ExternalInput")
    d_att_i = nc.dram_tensor("att_i", [E, 64], f32r, kind="ExternalInput")
    d_att_b1 = nc.dram_tensor("att_b1", [64, 1], f32, kind="ExternalInput")
    d_att_w2 = nc.dram_tensor("att_w2", [64, 1], f32r, kind="ExternalInput")
    d_fc_w1 = nc.dram_tensor("fc_w1", [H, 32], f32, kind="ExternalInput")
    d_fc_b1 = nc.dram_tensor("fc_b1", [32, 1], f32, kind="ExternalInput")
    d_fc_w2 = nc.dram_tensor("fc_w2", [32, 1], f32, kind="ExternalInput")
    d_fc_b2 = nc.dram_tensor("fc_b2", [1, 1], f32, kind="ExternalInput")
    d_ident = nc.dram_tensor("ident", [128, 128], f32, kind="ExternalInput")
    d_ones10 = nc.dram_tensor("ones10", [P, 1], f32, kind="ExternalInput")

    d_scr_w = nc.dram_tensor("scr_w", [P, bloc], f32, kind="Internal")
    d_scr_r = nc.dram_tensor("scr_r", [1, bloc], f32, kind="Internal")

    d_pred = nc.dram_tensor("pred_out", [1, bloc], f32, kind="ExternalOutput")
    d_wout = nc.dram_tensor("w_out", [P, bloc], f32, kind="ExternalOutput")

    def r32(ap):
        if ap.dtype == f32r:
            return ap
        return ap.bitcast(f32r)

    def pbcast(dram_ap, row, nparts):
        """AP reading one DRAM row, broadcast across nparts partitions."""
        sl = dram_ap[row:row + 1, :]
        return bass.AP(tensor=sl.tensor, offset=sl.offset,
                       ap=[[0, nparts]] + [list(d) for d in sl.ap[1:]])

    with tile.TileContext(nc) as tc, ExitStack() as top:
        const = top.enter_context(tc.tile_pool(name="const", bufs=1))

        def load(dram, shape, dtype=f32, name=None):
            t = const.tile(shape, dtype, name=name or (dram.name + "_sb"))
            nc.sync.dma_start(out=t[:], in_=dram.ap())
            return t

        w_node = load(d_w_node, [E, 4 * H], f32r)
        g_combo = load(d_g_combo, [16, 4 * H], f32r)
        w_hh = load(d_w_hh, [H, 4 * H], f32r)
        att_p = load(d_att_p, [H, 64], f32r)
        att_u = load(d_att_u, [E, 64], f32r)
        att_i = load(d_att_i, [E, 64], f32r)
        att_b1 = load(d_att_b1, [64, 1])
        att_w2 = load(d_att_w2, [64, 1], f32r)
        fc_w1 = load(d_fc_w1, [H, 32])
        fc_b1 = load(d_fc_b1, [32, 1])
        fc_w2 = load(d_fc_w2, [32, 1])
        fc_b2 = load(d_fc_b2, [1, 1])
        ident = load(d_ident, [128, 128])
        ones10 = load(d_ones10, [P, 1])
        idx_node = load(d_idx_node, [128, NCHUNK * GCOLS], i32)
        idx_user = load(d_idx_user, [128, bloc // 128], i32)
        idx_item = load(d_idx_item, [128, bloc // 128], i32)

        h_all = const.tile([H, ROWS], f32, name="h_all")
        scores_all = const.tile([P, bloc], f32, name="scores_all")
        user_eT = const.tile([E, bloc], f32r, name="user_eT")
        item_eT = const.tile([E, bloc], f32r, name="item_eT")

        import contextlib
        loop_cm = tc.For_i(0, reps, 1) if reps > 1 else contextlib.nullcontext()
        with loop_cm, ExitStack() as body:
            pg_pool = body.enter_context(
                tc.tile_pool(name="pg", bufs=2, space="PSUM"))
            gat_pool = body.enter_context(tc.tile_pool(name="gat", bufs=3))
            net_pool = body.enter_context(tc.tile_pool(name="net", bufs=2))
            oh_pool = body.enter_context(tc.tile_pool(name="oh", bufs=2))
            act_pool = body.enter_context(tc.tile_pool(name="actp", bufs=2))
            sc_pool = body.enter_context(tc.tile_pool(name="scp", bufs=2))
            t_sb = body.enter_context(tc.tile_pool(name="tsb", bufs=1))
            wb_pool = body.enter_context(tc.tile_pool(name="wbp", bufs=2))

            # ---- context (user/item) embeddings, transposed --------------
            for idx_t, dest in ((idx_user, user_eT), (idx_item, item_eT)):
                g = gat_pool.tile([128, bloc], f32, tag="gt", name="ctxg")
                nc.gpsimd.indirect_dma_start(
                    out=g[:], out_offset=None, in_=d_table.ap(),
                    in_offset=IndirectOffsetOnAxis(ap=idx_t[:, :], axis=0))
                for o, n in nsplit(bloc):  # per 512 cols of dest
                    tps = pg_pool.tile([128, n], f32, tag="pg", name="ctps")
                    for j in range(n // 128):
                        nc.tensor.transpose(
                            out=tps[:, j * 128:(j + 1) * 128],
                            in_=g[:, o + j * 128:o + (j + 1) * 128],
                            identity=ident[:])
                    nc.vector.tensor_copy(out=dest[:, o:o + n], in_=tps[:])

            # ---- main chunk loop -----------------------------------------
            for c in range(NCHUNK):
                p_idx = c // CPP
                b0 = (c % CPP) * chunk
                r0 = c * chunk

                g = gat_pool.tile([128, GCOLS * 128], f32, tag="gt", name="g")
                nc.gpsimd.indirect_dma_start(
                    out=g[:], out_offset=None, in_=d_table.ap(),
                    in_offset=IndirectOffsetOnAxis(
                        ap=idx_node[:, c * GCOLS:(c + 1) * GCOLS], axis=0))

                oh = oh_pool.tile([16, CW], f32r, name="oh")
                nc.sync.dma_start(out=oh[:],
                                  in_=d_onehot.ap()[:, c * CW:(c + 1) * CW])

                tps = pg_pool.tile([128, GCOLS * 128], f32, tag="pg",
                                   name="tps")
                for j in range(GCOLS):
                    nc.tensor.transpose(out=tps[:, j * 128:(j + 1) * 128],
                                        in_=g[:, j * 128:(j + 1) * 128],
                                        identity=ident[:])
                net = net_pool.tile([128, CW], f32r, name="net")
                nc.vector.tensor_copy(out=net[:], in_=tps[:])

                h_prev = None
                c_prev = None
                for l in range(3):
                    pg = pg_pool.tile([128, 4 * chunk], f32, tag="pg",
                                      name="pgl")
                    for m in range(4):
                        o_sl = pg[:, m * chunk:(m + 1) * chunk]
                        nc.tensor.matmul(
                            out=o_sl,
                            lhsT=w_node[:, m * 128:(m + 1) * 128],
                            rhs=net[:, l * chunk:(l + 1) * chunk],
                            start=True, stop=False)
                        nc.tensor.matmul(
                            out=o_sl,
                            lhsT=g_combo[:, m * 128:(m + 1) * 128],
                            rhs=oh[:, l * chunk:(l + 1) * chunk],
                            start=False, stop=(l == 0))
                        if l > 0:
                            nc.tensor.matmul(
                                out=o_sl,
                                lhsT=w_hh[:, m * 128:(m + 1) * 128],
                                rhs=r32(h_prev),
                                start=False, stop=True)

                    sio = act_pool.tile([128, 3 * chunk], f32, tag="sio",
                                        name="sio")
                    nc.scalar.activation(out=sio[:], in_=pg[:, 0:3 * chunk],
                                         func=AF.Sigmoid)
                    tg = act_pool.tile([128, chunk], f32, tag="tg", name="tg")
                    nc.scalar.activation(out=tg[:],
                                         in_=pg[:, 3 * chunk:4 * chunk],
                                         func=AF.Tanh)

                    c_new = act_pool.tile([128, chunk], f32, tag="cc",
                                          name="cc")
                    if l == 0:
                        nc.vector.tensor_mul(c_new[:], sio[:, 0:chunk], tg[:])
                    else:
                        t1 = act_pool.tile([128, chunk], f32, tag="t1",
                                           name="t1")
                        nc.vector.tensor_mul(t1[:], sio[:, 0:chunk], tg[:])
                        t2 = act_pool.tile([128, chunk], f32, tag="t2",
                                           name="t2")
                        nc.gpsimd.tensor_tensor(
                            out=t2[:], in0=sio[:, chunk:2 * chunk],
                            in1=c_prev[:], op=MUL)
                        nc.vector.tensor_add(c_new[:], t1[:], t2[:])

                    tc_t = act_pool.tile([128, chunk], f32, tag="tc",
                                         name="tct")
                    nc.scalar.activation(out=tc_t[:], in_=c_new[:],
                                         func=AF.Tanh)
                    if l == 2:
                        h_dst = h_all[:, r0:r0 + chunk]
                    else:
                        h_new = act_pool.tile([128, chunk], f32, tag="hh",
                                              name="hnew")
                        h_dst = h_new[:]
                    # round-to-f32r on write: h feeds f32r matmuls
                    nc.vector.tensor_mul(r32(h_dst),
                                         sio[:, 2 * chunk:3 * chunk],
                                         tc_t[:])
                    h_prev = h_dst
                    c_prev = c_new

                # ---- attention scores for this chunk ---------------------
                att_ps = pg_pool.tile([64, 2 * chunk], f32, tag="pg",
                                      name="attps")
                s1 = att_ps[:, 0:chunk]
                nc.tensor.matmul(out=s1, lhsT=att_p[:],
                                 rhs=r32(h_all[:, r0:r0 + chunk]),
                                 start=True, stop=False)
                nc.tensor.matmul(out=s1, lhsT=att_u[:],
                                 rhs=user_eT[:, b0:b0 + chunk],
                                 start=False, stop=False)
                nc.tensor.matmul(out=s1, lhsT=att_i[:],
                                 rhs=item_eT[:, b0:b0 + chunk],
                                 start=False, stop=True)
                r1 = sc_pool.tile([64, chunk], f32r, tag="r1", name="r1")
                nc.scalar.activation(out=r1[:], in_=s1, func=AF.Relu,
                                     bias=att_b1[:, 0:1])
                nc.tensor.matmul(out=att_ps[0:1, chunk:2 * chunk],
                                 lhsT=att_w2[:], rhs=r1[:],
                                 start=True, stop=True)
                sc = sc_pool.tile([1, chunk], f32, tag="sc", name="sc")
                nc.scalar.copy(out=sc[:], in_=att_ps[0:1, chunk:2 * chunk])
                nc.sync.dma_start(
                    out=scores_all[p_idx:p_idx + 1, b0:b0 + chunk], in_=sc[:])

            # ---- softmax over paths + weighted aggregation (full fp32) ---
            exp_sb = t_sb.tile([P, bloc], f32, name="exp_sb")
            nc.scalar.activation(out=exp_sb[:], in_=scores_all[:],
                                 func=AF.Exp)
            sum_ps = pg_pool.tile([1, bloc], f32, tag="pg", name="sum_ps")
            for o, n in nsplit(bloc):
                nc.tensor.matmul(out=sum_ps[:, o:o + n], lhsT=ones10[:],
                                 rhs=exp_sb[:, o:o + n],
                                 start=True, stop=True)
            recip = t_sb.tile([1, bloc], f32, name="recip")
            nc.vector.reciprocal(out=recip[:], in_=sum_ps[:])
            nc.sync.dma_start(out=d_scr_r.ap(), in_=recip[:])
            rb_sb = t_sb.tile([P, bloc], f32, name="rb_sb")
            nc.gpsimd.dma_start(out=rb_sb[:], in_=pbcast(d_scr_r.ap(), 0, P))
            w_sb = t_sb.tile([P, bloc], f32, name="w_sb")
            nc.vector.tensor_mul(w_sb[:], exp_sb[:], rb_sb[:])
            nc.sync.dma_start(out=d_wout.ap(), in_=w_sb[:])
            nc.sync.dma_start(out=d_scr_w.ap(), in_=w_sb[:])

            agg = t_sb.tile([H, bloc], f32, name="agg")
            tmp = t_sb.tile([H, bloc], f32, name="tmp")
            for p in range(P):
                wb = wb_pool.tile([128, bloc], f32, tag="wb", name="wb")
                nc.gpsimd.dma_start(out=wb[:],
                                    in_=pbcast(d_scr_w.ap(), p, 128))
                if p == 0:
                    nc.vector.tensor_mul(agg[:],
                                         h_all[:, p * bloc:(p + 1) * bloc],
                                         wb[:])
                else:
                    nc.vector.tensor_mul(tmp[:],
                                         h_all[:, p * bloc:(p + 1) * bloc],
                                         wb[:])
                    nc.vector.tensor_add(agg[:], agg[:], tmp[:])

            # ---- output MLP (full fp32 matmuls) --------------------------
            o1 = pg_pool.tile([32, bloc], f32, tag="pg", name="o1")
            for o, n in nsplit(bloc):
                nc.tensor.matmul(out=o1[:, o:o + n], lhsT=fc_w1[:],
                                 rhs=agg[:, o:o + n], start=True, stop=True)
            r1f = t_sb.tile([32, bloc], f32, name="r1f")
            nc.scalar.activation(out=r1f[:], in_=o1[:], func=AF.Relu,
                                 bias=fc_b1[:, 0:1])
            o2 = pg_pool.tile([1, bloc], f32, tag="pg", name="o2")
            for o, n in nsplit(bloc):
                nc.tensor.matmul(out=o2[:, o:o + n], lhsT=fc_w2[:],
                                 rhs=r1f[:, o:o + n], start=True, stop=True)
            pred_sb = t_sb.tile([1, bloc], f32, name="pred_sb")
            nc.scalar.add(out=pred_sb[:], in_=o2[:], add=fc_b2[:, 0:1])
            nc.sync.dma_start(out=d_pred.ap(), in_=pred_sb[:])

    nc.compile()
    _BUILD_CACHE[key] = nc
    return nc


# --------------------------------------------------------------------------
# host-side preparation
# --------------------------------------------------------------------------
def prepare_in_maps(inputs, ncores=NCORES, bloc=1024, chunk=512):
    f32 = np.float32
    RT = chunk // 128
    NCHUNK = (P * bloc) // chunk
    GCOLS = 3 * RT
    CW = 3 * chunk
    ROWS = P * bloc

    user = np.asarray(inputs["user"]).astype(np.int64)
    item = np.asarray(inputs["item"]).astype(np.int64)
    nodes = np.asarray(inputs["path_nodes"]).astype(np.int64)
    types = np.asarray(inputs["path_types"]).astype(np.int64)
    rels = np.asarray(inputs["path_rels"]).astype(np.int64)

    user_emb = np.asarray(inputs["user_emb"], dtype=f32)
    item_emb = np.asarray(inputs["item_emb"], dtype=f32)
    table = np.ascontiguousarray(np.concatenate([user_emb, item_emb], axis=0))

    W_ih = np.asarray(inputs["W_ih"], dtype=np.float64)
    W_hh = np.asarray(inputs["W_hh"], dtype=np.float64)
    bias = (np.asarray(inputs["b_ih"], np.float64)
            + np.asarray(inputs["b_hh"], np.float64))
    rel_emb = np.asarray(inputs["rel_emb"], np.float64)
    type_emb = np.asarray(inputs["type_emb"], np.float64)

    perm = np.r_[0:128, 128:256, 384:512, 256:384]  # [i, f, o, g]
    w_node = np.ascontiguousarray(W_ih[perm, 0:128].T).astype(f32)
    w_hh_l = np.ascontiguousarray(W_hh[perm, :].T).astype(f32)
    G = np.zeros((16, 4 * H), np.float64)
    for s in range(16):
        r_, t_ = s >> 1, s & 1
        G[s] = (rel_emb[r_] @ W_ih[perm, 256:384].T
                + type_emb[t_] @ W_ih[perm, 128:256].T + bias[perm])
    g_combo = G.astype(f32)

    att_w1 = np.asarray(inputs["att_w1"], dtype=f32)
    att_b1 = np.asarray(inputs["att_b1"], dtype=f32).reshape(64, 1)
    att_w2 = np.asarray(inputs["att_w2"], dtype=f32).reshape(64, 1)
    fc_w1 = np.asarray(inputs["fc_w1"], dtype=f32)
    fc_b1 = np.asarray(inputs["fc_b1"], dtype=f32).reshape(32, 1)
    fc_w2 = np.asarray(inputs["fc_w2"], dtype=f32).reshape(32, 1)
    fc_b2 = np.asarray(inputs["fc_b2"], dtype=f32).reshape(1, 1)
    ident = np.eye(128, dtype=f32)
    ones10 = np.ones((P, 1), f32)

    cidx = (nodes + (types == 1) * (NU + 1)).astype(np.int32)  # [B, P, L]
    s_code = (2 * rels + types).astype(np.int64)               # [B, P, L]

    shared = dict(table=table, w_node=w_node, g_combo=g_combo, w_hh=w_hh_l,
                  att_p=np.ascontiguousarray(att_w1[0:128]),
                  att_u=np.ascontiguousarray(att_w1[128:256]),
                  att_i=np.ascontiguousarray(att_w1[256:384]),
                  att_b1=att_b1, att_w2=att_w2, fc_w1=fc_w1, fc_b1=fc_b1,
                  fc_w2=fc_w2, fc_b2=fc_b2, ident=ident, ones10=ones10)

    in_maps = []
    for k in range(ncores):
        bsl = slice(k * bloc, (k + 1) * bloc)
        # r-major (r = p*bloc + b) index/selector arrays
        ci_r = cidx[bsl].transpose(1, 0, 2).reshape(ROWS, L)
        s_r = s_code[bsl].transpose(1, 0, 2).reshape(ROWS, L)

        v = ci_r.reshape(NCHUNK, RT, 128, L)
        idx_node = np.ascontiguousarray(
            v.transpose(2, 0, 3, 1).reshape(128, NCHUNK * GCOLS))

        s_v = s_r.reshape(NCHUNK, chunk, L)
        s_cols = s_v.transpose(0, 2, 1).reshape(NCHUNK * CW)
        onehot = (np.arange(16)[:, None] == s_cols[None, :]).astype(f32)

        idx_user = np.ascontiguousarray(
            user[bsl].reshape(bloc // 128, 128).T.astype(np.int32))
        idx_item = np.ascontiguousarray(
            (item[bsl] + (NU + 1)).reshape(bloc // 128, 128).T
            .astype(np.int32))

        m = dict(shared)
        m.update(idx_node=idx_node, idx_user=idx_user, idx_item=idx_item,
                 onehot=onehot)
        in_maps.append(m)
    return in_maps


def assemble_outputs(results, ncores=NCORES, bloc=1024):
    pred = np.concatenate(
        [results[k]["pred_out"][0] for k in range(ncores)])[:, None]
    weights = np.concatenate(
        [results[k]["w_out"].T for k in range(ncores)], axis=0)[:, :, None]
    return pred.astype(np.float32), weights.astype(np.float32)


# --------------------------------------------------------------------------
# entry point
# --------------------------------------------------------------------------
def kernel(**inputs):
    nc = build_program(1024, 512, NCORES)
    in_maps = prepare_in_maps(inputs, NCORES, 1024, 512)
    from concourse.bass_utils import run_bass_kernel_spmd
    res = run_bass_kernel_spmd(nc, in_maps, core_ids=list(range(NCORES)))
    return assemble_outputs(res.results, NCORES, 1024)
